# revision 28
# baseline (speedup 1.0000x reference)
"""CondLaneHead Trainium2 kernel.

Takes FULL inputs (as produced by the reference setup_inputs), shards over
8 NeuronCores (4 images per core), runs a Bass/Tile kernel per core, and
reassembles the full outputs on the host.

Per-core device program (per image, fully symmetric):
  - heatmap branch: 3x3 conv (Cin=256 -> 64) + ReLU, then 1x1 conv (64 -> 1)
  - mask branch:    3x3 conv (Cin=128 -> 64) + ReLU -> mf
  - dynamic heads:  matmul over mf channels (K=64) plus a K=3 matmul for
    the CoordConv x/y/bias term, accumulated in the same PSUM bank.
Instance routing (which image each of the 128 instances reads) is resolved
on the host: dynamic-conv params are packed into per-image weight slots.

Convs are evaluated as 9 (resp. 18) shifted matmuls accumulating in PSUM
over 500-pixel chunks of the zero-padded image (padding is baked in on the
host so every DMA is contiguous). Matmuls run in float32r (TF32-like,
~1.4e-4 rel err, 4x the fp32 MAC rate); float32r only supports PE column
position 0, so images are processed serially at M=64.
"""

import os
import sys

sys.path.insert(0, "/opt/trn_rl_repo")

import numpy as np

import concourse.bass as bass
import concourse.bacc as bacc
import concourse.mybir as mybir
from concourse import tile
from concourse.bass_utils import run_bass_kernel_spmd

# ---- problem constants (hardcoded per spec) ----
B = 32
H16, W16 = 20, 50  # heatmap spatial
H8, W8 = 40, 100   # mask spatial
CH = 64            # head conv width (both branches' mid channels)
NCORES = 8
IPC = B // NCORES  # images per core = 4

PH16, PW16 = H16 + 2, W16 + 2        # 22, 52  padded
PH8, PW8 = H8 + 2, W8 + 2            # 42, 102 padded
PHW16 = PH16 * PW16                  # 1144
PHW8 = PH8 * PW8                     # 4284
HW16 = H16 * W16                     # 1000
HW8 = H8 * W8                        # 4000
CHK = 500                            # free-dim chunk (<=512 fp32 / 1 psum bank)
NC16 = HW16 // CHK                   # 2
NC8 = HW8 // CHK                     # 8
PSB = 512                            # psum bank width in fp32
R16 = CHK // W16                     # 10 output rows per chunk
R8 = CHK // W8                       # 5
TAPS = [(ky, kx) for ky in range(3) for kx in range(3)]

F32 = mybir.dt.float32
# float32r = TF32-like matmul mode: 4x faster than fp32 on the PE at
# free-dim >= 256, data stays fp32 in memory, ~1.4e-4 rel err.
MM_DTYPE = {"f32": mybir.dt.float32, "f32r": mybir.dt.float32r}[
    os.environ.get("KERNEL_MM_DTYPE", "f32r")
]
# "f32r": serial images, all-f32r matmuls (~1.2e-4 e2e rel err)
# "bf16": bf16 convs with image-pair column packing + f32r dynamic heads
#         (~2.5e-3 e2e rel err, ~2x faster)
VARIANT = os.environ.get("KERNEL_VARIANT", "f32r")

_PROGRAM_CACHE: dict = {}


def _build(slot: int, mm_dtype) -> bass.Bass:
    """Build the single-core Bass program (shared by all 8 cores)."""
    DT = mm_dtype
    nc = bacc.Bacc("TRN2", target_bir_lowering=False, debug=False)

    fhm_d = nc.declare_dram_parameter("fhm", [IPC, 2, 128, PHW16], DT, isOutput=False)
    fm_d = nc.declare_dram_parameter("fmask", [IPC, 128, PHW8], DT, isOutput=False)
    w1t_d = nc.declare_dram_parameter("w1t", [128, 18 * CH], DT, isOutput=False)
    wmt_d = nc.declare_dram_parameter("wmt", [128, 9 * CH], DT, isOutput=False)
    w2_d = nc.declare_dram_parameter("w2", [CH, 1], DT, isOutput=False)
    coords_d = nc.declare_dram_parameter("coords", [3, HW8], DT, isOutput=False)
    wmain_d = nc.declare_dram_parameter("wmain", [67, IPC * slot], DT, isOutput=False)
    bias_h_d = nc.declare_dram_parameter("bias_h", [CH, 1], F32, isOutput=False)
    bias_mf_d = nc.declare_dram_parameter("bias_mf", [CH, 1], F32, isOutput=False)

    hm_out_d = nc.declare_dram_parameter("hm_out", [IPC, HW16], F32, isOutput=True)
    dyn_out_d = nc.declare_dram_parameter("dyn_out", [IPC, slot, HW8], F32, isOutput=True)

    RELU = mybir.ActivationFunctionType.Relu

    with tile.TileContext(nc) as tc:
        with (
            tc.tile_pool(name="const", bufs=1) as constp,
            tc.tile_pool(name="fin", bufs=2) as finp,
            tc.tile_pool(name="mf", bufs=2) as mfp,
            tc.tile_pool(name="h", bufs=2) as hp,
            tc.tile_pool(name="stage", bufs=2) as stagep,
            tc.tile_pool(name="cps", bufs=5, space="PSUM") as cpsp,
            tc.tile_pool(name="dps", bufs=2, space="PSUM") as dpsp,
            tc.tile_pool(name="hps", bufs=1, space="PSUM") as hpsp,
        ):
            # ---- constants into SBUF ----
            w1t_s = constp.tile([128, 18 * CH], DT)
            nc.sync.dma_start(w1t_s[:], w1t_d[:])
            wmt_s = constp.tile([128, 9 * CH], DT)
            nc.sync.dma_start(wmt_s[:], wmt_d[:])
            w2_s = constp.tile([CH, 1], DT)
            nc.sync.dma_start(w2_s[:], w2_d[:])
            wmain_s = constp.tile([67, IPC * slot], DT)
            nc.sync.dma_start(wmain_s[:], wmain_d[:])
            bias_h_s = constp.tile([CH, 1], F32)
            nc.sync.dma_start(bias_h_s[:], bias_h_d[:])
            bias_mf_s = constp.tile([CH, 1], F32)
            nc.sync.dma_start(bias_mf_s[:], bias_mf_d[:])

            for il in range(IPC):
                # ---- load padded inputs for the image ----
                fhm_t = finp.tile([128, 2 * PHW16], DT, tag="fhm")
                for j in range(2):
                    nc.sync.dma_start(
                        fhm_t[:, j * PHW16:(j + 1) * PHW16], fhm_d[il, j]
                    )
                fm_t = finp.tile([128, PHW8], DT, tag="fm")
                nc.sync.dma_start(fm_t[:], fm_d[il])

                fhm_v = fhm_t[:].rearrange("q (b r c) -> q b r c", b=2, r=PH16)
                fm_v = fm_t[:].rearrange("q (r c) -> q r c", r=PH8)

                # ---- heatmap branch: 3x3 conv + ReLU ----
                h_t = hp.tile([CH, HW16], DT)
                for c in range(NC16):
                    ps = cpsp.tile([128, PSB], F32, tag="cps")
                    for j in range(2):  # cin chunks
                        for t, (ky, kx) in enumerate(TAPS):
                            rhs = fhm_v[
                                :, j,
                                R16 * c + ky: R16 * c + ky + R16,
                                kx: kx + W16,
                            ]
                            nc.tensor.matmul(
                                ps[:CH, :CHK],
                                w1t_s[:, (j * 9 + t) * CH:(j * 9 + t + 1) * CH],
                                rhs,
                                start=(j == 0 and t == 0),
                                stop=(j == 1 and t == 8),
                            )
                    nc.scalar.activation(
                        h_t[:, c * CHK:(c + 1) * CHK],
                        ps[:CH, :CHK],
                        RELU,
                        bias=bias_h_s[:],
                    )

                # ---- heatmap branch: 1x1 conv ----
                hm_sb = stagep.tile([1, HW16], F32, tag="hmsb")
                for c in range(NC16):
                    hps_t = hpsp.tile([128, PSB], F32, tag="hps")
                    nc.tensor.matmul(
                        hps_t[:1, :CHK],
                        w2_s[:],
                        h_t[:, c * CHK:(c + 1) * CHK],
                        start=True,
                        stop=True,
                    )
                    nc.vector.tensor_copy(
                        hm_sb[:, c * CHK:(c + 1) * CHK], hps_t[:1, :CHK]
                    )
                nc.sync.dma_start(hm_out_d[il:il + 1, :], hm_sb[:])

                # ---- mask branch: 3x3 conv + ReLU ----
                # rows 0-63: mf; rows 64-66: coord_x / coord_y / ones, so the
                # dynamic head is a single K=67 contraction.
                mf_t = mfp.tile([67, HW8], DT)
                nc.sync.dma_start(mf_t[64:67, :], coords_d[:])
                for c in range(NC8):
                    ps = cpsp.tile([128, PSB], F32, tag="cps")
                    for t, (ky, kx) in enumerate(TAPS):
                        rhs = fm_v[
                            :,
                            R8 * c + ky: R8 * c + ky + R8,
                            kx: kx + W8,
                        ]
                        nc.tensor.matmul(
                            ps[:CH, :CHK],
                            wmt_s[:, t * CH:(t + 1) * CH],
                            rhs,
                            start=(t == 0),
                            stop=(t == 8),
                        )
                    nc.scalar.activation(
                        mf_t[:CH, c * CHK:(c + 1) * CHK],
                        ps[:CH, :CHK],
                        RELU,
                        bias=bias_mf_s[:],
                    )

                # ---- dynamic heads (single K=67 matmul per chunk) ----
                dyn_sb = stagep.tile([slot, HW8], F32, tag="dynsb")
                for c in range(NC8):
                    dps_t = dpsp.tile([128, PSB], F32, tag="dps")
                    nc.tensor.matmul(
                        dps_t[:slot, :CHK],
                        wmain_s[:, il * slot:(il + 1) * slot],
                        mf_t[:, c * CHK:(c + 1) * CHK],
                        start=True,
                        stop=True,
                    )
                    nc.vector.tensor_copy(
                        dyn_sb[:, c * CHK:(c + 1) * CHK], dps_t[:slot, :CHK]
                    )
                nc.sync.dma_start(dyn_out_d[il], dyn_sb[:, :])

    nc.finalize()
    return nc


def _build_bf16(slot: int) -> bass.Bass:
    """bf16 variant: image pairs packed into PE column halves (col tiling,
    concurrent matmuls) for convs AND dynamic heads. All matmul data bf16
    (error ~4e-3 rel, dominated by input rounding on the heatmap branch);
    PSUM accumulation stays fp32.

    Schedule per pair: heatmap conv first (its input is smaller and lands
    first), then mask conv with lag-1 pipelined dynamic heads.
    """
    BF = mybir.dt.bfloat16
    nc = bacc.Bacc("TRN2", target_bir_lowering=False, debug=False)

    fhm_d = nc.declare_dram_parameter("fhm", [IPC, 2, 128, PHW16], BF, isOutput=False)
    fm_d = nc.declare_dram_parameter("fmask", [IPC, 128, PHW8], BF, isOutput=False)
    w1t_d = nc.declare_dram_parameter("w1t", [128, 18 * CH], BF, isOutput=False)
    wmt_d = nc.declare_dram_parameter("wmt", [128, 9 * CH], BF, isOutput=False)
    w2_d = nc.declare_dram_parameter("w2", [128, 1], BF, isOutput=False)
    coords_d = nc.declare_dram_parameter("coords", [3, HW8], BF, isOutput=False)
    wmain_d = nc.declare_dram_parameter("wmain", [67, IPC * slot], BF, isOutput=False)
    bias_h_d = nc.declare_dram_parameter("bias_h", [CH, 1], F32, isOutput=False)
    bias_mf_d = nc.declare_dram_parameter("bias_mf", [CH, 1], F32, isOutput=False)

    hm_out_d = nc.declare_dram_parameter("hm_out", [IPC, HW16], F32, isOutput=True)
    dyn_out_d = nc.declare_dram_parameter("dyn_out", [IPC, slot, HW8], F32, isOutput=True)

    RELU = mybir.ActivationFunctionType.Relu

    with tile.TileContext(nc) as tc:
        with (
            tc.tile_pool(name="const", bufs=1) as constp,
            tc.tile_pool(name="fin", bufs=2) as finp,
            tc.tile_pool(name="mf", bufs=2) as mfp,
            tc.tile_pool(name="h", bufs=2) as hp,
            tc.tile_pool(name="stage", bufs=2) as stagep,
            tc.tile_pool(name="cps", bufs=4, space="PSUM") as cpsp,
            tc.tile_pool(name="dps", bufs=3, space="PSUM") as dpsp,
            tc.tile_pool(name="hps", bufs=1, space="PSUM") as hpsp,
        ):
            # first pair's heatmap input leads the sync ring; weights ride
            # the scalar ring concurrently.
            fhm_tiles, fm_tiles, h_tiles = [], [], []
            for p in range(2):
                fhm_t = finp.tile([128, 4 * PHW16], BF, name=f"fhm{p}", tag="fhm")
                fm_t = finp.tile([128, 2 * PHW8], BF, name=f"fm{p}", tag="fm")
                fhm_tiles.append(fhm_t)
                fm_tiles.append(fm_t)
            w1t_s = constp.tile([128, 18 * CH], BF)
            nc.sync.dma_start(w1t_s[:], w1t_d[:])
            nc.scalar.dma_start(
                fhm_tiles[0][:].rearrange("q (b f) -> q b f", b=4),
                fhm_d[0:2].rearrange("a b q f -> q (a b) f"),
            )
            nc.scalar.dma_start(
                fhm_tiles[1][:].rearrange("q (b f) -> q b f", b=4),
                fhm_d[2:4].rearrange("a b q f -> q (a b) f"),
            )
            nc.sync.dma_start(
                fm_tiles[0][:].rearrange("q (i f) -> q i f", i=2),
                fm_d[0:2].rearrange("a q f -> q a f"),
            )
            nc.sync.dma_start(
                fm_tiles[1][:].rearrange("q (i f) -> q i f", i=2),
                fm_d[2:4].rearrange("a q f -> q a f"),
            )
            wmt_s = constp.tile([128, 9 * CH], BF)
            nc.sync.dma_start(wmt_s[:], wmt_d[:])
            w2_s = constp.tile([128, 1], BF)
            nc.scalar.dma_start(w2_s[:], w2_d[:])
            wmain_s = constp.tile([67, IPC * slot], BF)
            nc.scalar.dma_start(wmain_s[:], wmain_d[:])
            bias_h_s = constp.tile([CH, 1], F32)
            nc.scalar.dma_start(bias_h_s[:], bias_h_d[:])
            bias_mf_s = constp.tile([CH, 1], F32)
            nc.scalar.dma_start(bias_mf_s[:], bias_mf_d[:])

            for p in range(2):
                # ---- heatmap branch: 3x3 conv + ReLU, then 1x1 ----
                fhm_v = fhm_tiles[p][:].rearrange("q (b r c) -> q b r c", b=4, r=PH16)
                h_t = hp.tile([128, HW16], BF)
                for c in range(NC16):
                    ps2 = [cpsp.tile([128, PSB], F32, name=f"cps{i_}", tag="cps")
                           for i_ in range(2)]
                    for j in range(2):
                        for t, (ky, kx) in enumerate(TAPS):
                            for i in range(2):
                                rhs = fhm_v[
                                    :, i * 2 + j,
                                    R16 * c + ky: R16 * c + ky + R16,
                                    kx: kx + W16,
                                ]
                                nc.tensor.matmul(
                                    ps2[i][i * CH:(i + 1) * CH, :CHK],
                                    w1t_s[:, (j * 9 + t) * CH:(j * 9 + t + 1) * CH],
                                    rhs,
                                    start=(j == 0 and t == 0),
                                    stop=(j == 1 and t == 8),
                                )
                    for i in range(2):
                        nc.scalar.activation(
                            h_t[i * CH:(i + 1) * CH, c * CHK:(c + 1) * CHK],
                            ps2[i][i * CH:(i + 1) * CH, :CHK],
                            RELU,
                            bias=bias_h_s[:],
                        )
                h_tiles.append(h_t)

            for p in range(2):
                # ---- heatmap 1x1 conv (h ReLUs finished in the hm phase) ----
                h_t = h_tiles[p]
                hm_sb = stagep.tile([64, HW16], F32, tag="hmsb")
                for c in range(NC16):
                    hps_t = hpsp.tile([128, PSB], F32, tag="hps")
                    for i in range(2):
                        nc.tensor.matmul(
                            hps_t[i * 32:i * 32 + 1, :CHK],
                            w2_s[i * CH:(i + 1) * CH, :],
                            h_t[i * CH:(i + 1) * CH, c * CHK:(c + 1) * CHK],
                            start=True,
                            stop=True,
                        )
                    for i in range(2):
                        nc.vector.tensor_copy(
                            hm_sb[i * 32:i * 32 + 1, c * CHK:(c + 1) * CHK],
                            hps_t[i * 32:i * 32 + 1, :CHK],
                        )
                for i in range(2):
                    nc.sync.dma_start(
                        hm_out_d[2 * p + i:2 * p + i + 1, :],
                        hm_sb[i * 32:i * 32 + 1, :],
                    )

                # ---- mask branch: 3x3 conv + ReLU ----
                # per-image mf tiles: rows 0-63 mf, rows 64-66 coords, so the
                # dynamic head is one K=67 matmul. Image B's ReLU shifts
                # partitions 64-127 -> 0-63.
                fm_v = fm_tiles[p][:].rearrange("q (b r c) -> q b r c", b=2, r=PH8)
                mf2 = []
                for i in range(2):
                    mf_t = mfp.tile([67, HW8], BF, name=f"mf{i}", tag=f"mf{i}")
                    nc.sync.dma_start(mf_t[64:67, :], coords_d[:])
                    mf2.append(mf_t)
                dyn_sb = stagep.tile([2 * slot, HW8], F32, tag="dynsb")

                def emit_dyn(c):
                    dps2 = [dpsp.tile([128, PSB], F32, name=f"dps{i_}", tag="dps")
                            for i_ in range(2)]
                    for i in range(2):
                        nc.tensor.matmul(
                            dps2[i][i * slot:(i + 1) * slot, :CHK],
                            wmain_s[:, (2 * p + i) * slot:(2 * p + i + 1) * slot],
                            mf2[i][:, c * CHK:(c + 1) * CHK],
                            start=True,
                            stop=True,
                        )
                    for i in range(2):
                        nc.vector.tensor_copy(
                            dyn_sb[i * slot:(i + 1) * slot, c * CHK:(c + 1) * CHK],
                            dps2[i][i * slot:(i + 1) * slot, :CHK],
                        )

                for c in range(NC8):
                    ps2 = [cpsp.tile([128, PSB], F32, name=f"cpsm{i_}", tag="cps")
                           for i_ in range(2)]
                    for t, (ky, kx) in enumerate(TAPS):
                        for i in range(2):
                            rhs = fm_v[
                                :, i,
                                R8 * c + ky: R8 * c + ky + R8,
                                kx: kx + W8,
                            ]
                            nc.tensor.matmul(
                                ps2[i][i * CH:(i + 1) * CH, :CHK],
                                wmt_s[:, t * CH:(t + 1) * CH],
                                rhs,
                                start=(t == 0),
                                stop=(t == 8),
                            )
                    for i in range(2):
                        nc.scalar.activation(
                            mf2[i][:CH, c * CHK:(c + 1) * CHK],
                            ps2[i][i * CH:(i + 1) * CH, :CHK],
                            RELU,
                            bias=bias_mf_s[:],
                        )
                    if c >= 1:
                        emit_dyn(c - 1)
                emit_dyn(NC8 - 1)
                for i in range(2):
                    nc.sync.dma_start(
                        dyn_out_d[2 * p + i], dyn_sb[i * slot:(i + 1) * slot, :]
                    )

    nc.finalize()
    return nc


def _prepare(f_hm, f_mask, dyn_params, inst_idx, hm_w1, hm_b1, hm_w2, hm_b2,
             mask_w, mask_b, variant="f32r"):
    """Host-side sharding + packing. Returns (in_maps, slot, inst_map)."""
    f_hm = np.asarray(f_hm, np.float32)
    f_mask = np.asarray(f_mask, np.float32)
    dyn_params = np.asarray(dyn_params, np.float32)
    inst_idx = np.asarray(inst_idx).astype(np.int64)
    hm_w1 = np.asarray(hm_w1, np.float32)
    hm_b1 = np.asarray(hm_b1, np.float32)
    mask_w = np.asarray(mask_w, np.float32)
    mask_b = np.asarray(mask_b, np.float32)

    n_inst = inst_idx.shape[0]
    counts = np.bincount(inst_idx, minlength=B)
    kmax = int(counts.max())
    half = max(16, -(-kmax // 16) * 16)  # per-head slot count, multiple of 16
    slot = 2 * half
    if slot > 128:
        raise ValueError(f"too many instances on one image: {kmax}")

    # padded conv inputs (zero borders baked in -> contiguous DMAs)
    fhm_pad = np.zeros((B, 2, 128, PH16, PW16), np.float32)
    fhm_pad[:, :, :, 1:1 + H16, 1:1 + W16] = f_hm.reshape(B, 2, 128, H16, W16)
    fhm_pad = fhm_pad.reshape(NCORES, IPC, 2, 128, PHW16)
    fm_pad = np.zeros((B, 128, PH8, PW8), np.float32)
    fm_pad[:, :, 1:1 + H8, 1:1 + W8] = f_mask
    fm_pad = fm_pad.reshape(NCORES, IPC, 128, PHW8)

    # conv weights as lhsT tap matrices, packed in SBUF layout [K=128, taps*M]
    w1t = np.ascontiguousarray(
        hm_w1.reshape(CH, 2, 128, 3, 3).transpose(2, 1, 3, 4, 0).reshape(128, 18 * CH)
    )
    wmt = np.ascontiguousarray(
        mask_w.transpose(1, 2, 3, 0).reshape(128, 9 * CH)
    )
    w2 = np.ascontiguousarray(np.asarray(hm_w2, np.float32).reshape(1, CH).T)
    if variant == "bf16":
        import ml_dtypes
        bf = ml_dtypes.bfloat16
        fhm_pad = fhm_pad.astype(bf)
        fm_pad = fm_pad.astype(bf)
        w1t = w1t.astype(bf)
        wmt = wmt.astype(bf)
        w2 = np.concatenate([w2, w2], axis=0).astype(bf)

    coords = np.stack([
        np.tile(np.arange(W8, dtype=np.float32), H8),
        np.repeat(np.arange(H8, dtype=np.float32), W8),
        np.ones(HW8, np.float32),
    ])

    bias_h = hm_b1[:, None].astype(np.float32)
    bias_mf = mask_b[:, None].astype(np.float32)

    # dynamic params routed to owning (core, image) and packed into slots.
    # rows 0-63: mf-channel weights; rows 64-66: coord_x / coord_y / bias.
    wmain = np.zeros((NCORES, 67, IPC * slot), np.float32)
    inst_map = np.zeros((n_inst, 3), np.int64)  # (core, img_local, slot_i)
    for bimg in range(B):
        k, il = divmod(bimg, IPC)
        ns = np.nonzero(inst_idx == bimg)[0]
        for si, n in enumerate(ns):
            col = il * slot + si
            wmain[k, :CH, col] = dyn_params[n, 2:66]
            wmain[k, 64:67, col] = (
                dyn_params[n, 0], dyn_params[n, 1], dyn_params[n, 66] - 2.19
            )
            col = il * slot + half + si
            wmain[k, :CH, col] = dyn_params[n, 69:133]
            wmain[k, 64:67, col] = (
                dyn_params[n, 67], dyn_params[n, 68], dyn_params[n, 133]
            )
            inst_map[n] = (k, il, si)

    if variant == "bf16":
        import ml_dtypes
        coords = coords.astype(ml_dtypes.bfloat16)
        wmain = wmain.astype(ml_dtypes.bfloat16)
    in_maps = []
    for k in range(NCORES):
        in_maps.append({
            "fhm": np.ascontiguousarray(fhm_pad[k]),
            "fmask": np.ascontiguousarray(fm_pad[k]),
            "w1t": w1t,
            "wmt": wmt,
            "w2": w2,
            "coords": np.ascontiguousarray(coords),
            "wmain": np.ascontiguousarray(wmain[k]),
            "bias_h": bias_h,
            "bias_mf": bias_mf,
        })
    return in_maps, slot, inst_map


def _postprocess(results, slot, inst_map, hm_b2):
    half = slot // 2
    hm = np.stack([r["hm_out"] for r in results])          # [8, IPC, 1000]
    hm = hm.reshape(B, 1, H16, W16) + np.float32(np.asarray(hm_b2).reshape(()))
    n_inst = inst_map.shape[0]
    masks = np.empty((n_inst, 1, H8, W8), np.float32)
    regs = np.empty((n_inst, 1, H8, W8), np.float32)
    for n in range(n_inst):
        k, il, si = inst_map[n]
        dyn = results[k]["dyn_out"]                        # [IPC, slot, 4000]
        masks[n, 0] = dyn[il, si].reshape(H8, W8)
        regs[n, 0] = dyn[il, half + si].reshape(H8, W8)
    return hm.astype(np.float32), masks, regs


LAST_RESULT = None


def kernel(f_hm, f_mask, dyn_params, inst_idx, hm_w1, hm_b1, hm_w2, hm_b2,
           mask_w, mask_b):
    global LAST_RESULT
    in_maps, slot, inst_map = _prepare(
        f_hm, f_mask, dyn_params, inst_idx, hm_w1, hm_b1, hm_w2, hm_b2,
        mask_w, mask_b, variant=VARIANT,
    )
    key = (slot, VARIANT, MM_DTYPE)
    if key not in _PROGRAM_CACHE:
        if VARIANT == "bf16":
            _PROGRAM_CACHE[key] = _build_bf16(slot)
        else:
            _PROGRAM_CACHE[key] = _build(slot, MM_DTYPE)
    nc = _PROGRAM_CACHE[key]
    trace = bool(int(os.environ.get("KERNEL_TRACE", "0")))
    res = run_bass_kernel_spmd(nc, in_maps, list(range(NCORES)), trace=trace)
    LAST_RESULT = res
    return _postprocess(res.results, slot, inst_map, hm_b2)


# revision 30
# speedup vs baseline: 1.0277x; 1.0277x over previous
"""CondLaneHead Trainium2 kernel.

Takes FULL inputs (as produced by the reference setup_inputs), shards over
8 NeuronCores (4 images per core), runs a Bass/Tile kernel per core, and
reassembles the full outputs on the host.

Per-core device program (per image, fully symmetric):
  - heatmap branch: 3x3 conv (Cin=256 -> 64) + ReLU, then 1x1 conv (64 -> 1)
  - mask branch:    3x3 conv (Cin=128 -> 64) + ReLU -> mf
  - dynamic heads:  matmul over mf channels (K=64) plus a K=3 matmul for
    the CoordConv x/y/bias term, accumulated in the same PSUM bank.
Instance routing (which image each of the 128 instances reads) is resolved
on the host: dynamic-conv params are packed into per-image weight slots.

Convs are evaluated as 9 (resp. 18) shifted matmuls accumulating in PSUM
over 500-pixel chunks of the zero-padded image (padding is baked in on the
host so every DMA is contiguous). Matmuls run in float32r (TF32-like,
~1.4e-4 rel err, 4x the fp32 MAC rate); float32r only supports PE column
position 0, so images are processed serially at M=64.
"""

import os
import sys

sys.path.insert(0, "/opt/trn_rl_repo")

import numpy as np

import concourse.bass as bass
import concourse.bacc as bacc
import concourse.mybir as mybir
from concourse import tile
from concourse.bass_utils import run_bass_kernel_spmd

# ---- problem constants (hardcoded per spec) ----
B = 32
H16, W16 = 20, 50  # heatmap spatial
H8, W8 = 40, 100   # mask spatial
CH = 64            # head conv width (both branches' mid channels)
NCORES = 8
IPC = B // NCORES  # images per core = 4

PH16, PW16 = H16 + 2, W16 + 2        # 22, 52  padded
PH8, PW8 = H8 + 2, W8 + 2            # 42, 102 padded
PHW16 = PH16 * PW16                  # 1144
PHW8 = PH8 * PW8                     # 4284
HW16 = H16 * W16                     # 1000
HW8 = H8 * W8                        # 4000
CHK = 500                            # free-dim chunk (<=512 fp32 / 1 psum bank)
NC16 = HW16 // CHK                   # 2
NC8 = HW8 // CHK                     # 8
PSB = 512                            # psum bank width in fp32
R16 = CHK // W16                     # 10 output rows per chunk
R8 = CHK // W8                       # 5
TAPS = [(ky, kx) for ky in range(3) for kx in range(3)]

F32 = mybir.dt.float32
# float32r = TF32-like matmul mode: 4x faster than fp32 on the PE at
# free-dim >= 256, data stays fp32 in memory, ~1.4e-4 rel err.
MM_DTYPE = {"f32": mybir.dt.float32, "f32r": mybir.dt.float32r}[
    os.environ.get("KERNEL_MM_DTYPE", "f32r")
]
# "f32r": serial images, all-f32r matmuls (~1.2e-4 e2e rel err)
# "bf16": bf16 convs with image-pair column packing + f32r dynamic heads
#         (~2.5e-3 e2e rel err, ~2x faster)
VARIANT = os.environ.get("KERNEL_VARIANT", "f32r")

_PROGRAM_CACHE: dict = {}


def _build(slot: int, mm_dtype) -> bass.Bass:
    """Build the single-core Bass program (shared by all 8 cores)."""
    DT = mm_dtype
    nc = bacc.Bacc("TRN2", target_bir_lowering=False, debug=False)

    fhm_d = nc.declare_dram_parameter("fhm", [IPC, 2, 128, PHW16], DT, isOutput=False)
    fm_d = nc.declare_dram_parameter("fmask", [IPC, 128, PHW8], DT, isOutput=False)
    w1t_d = nc.declare_dram_parameter("w1t", [128, 18 * CH], DT, isOutput=False)
    wmt_d = nc.declare_dram_parameter("wmt", [128, 9 * CH], DT, isOutput=False)
    w2_d = nc.declare_dram_parameter("w2", [CH, 1], DT, isOutput=False)
    coords_d = nc.declare_dram_parameter("coords", [3, HW8], DT, isOutput=False)
    wmain_d = nc.declare_dram_parameter("wmain", [67, IPC * slot], DT, isOutput=False)
    bias_h_d = nc.declare_dram_parameter("bias_h", [CH, 1], F32, isOutput=False)
    bias_mf_d = nc.declare_dram_parameter("bias_mf", [CH, 1], F32, isOutput=False)

    hm_out_d = nc.declare_dram_parameter("hm_out", [IPC, HW16], F32, isOutput=True)
    dyn_out_d = nc.declare_dram_parameter("dyn_out", [IPC, slot, HW8], F32, isOutput=True)

    RELU = mybir.ActivationFunctionType.Relu

    with tile.TileContext(nc) as tc:
        with (
            tc.tile_pool(name="const", bufs=1) as constp,
            tc.tile_pool(name="fin", bufs=2) as finp,
            tc.tile_pool(name="mf", bufs=2) as mfp,
            tc.tile_pool(name="h", bufs=2) as hp,
            tc.tile_pool(name="stage", bufs=2) as stagep,
            tc.tile_pool(name="cps", bufs=5, space="PSUM") as cpsp,
            tc.tile_pool(name="dps", bufs=2, space="PSUM") as dpsp,
            tc.tile_pool(name="hps", bufs=1, space="PSUM") as hpsp,
        ):
            # ---- constants into SBUF ----
            w1t_s = constp.tile([128, 18 * CH], DT)
            nc.sync.dma_start(w1t_s[:], w1t_d[:])
            wmt_s = constp.tile([128, 9 * CH], DT)
            nc.sync.dma_start(wmt_s[:], wmt_d[:])
            w2_s = constp.tile([CH, 1], DT)
            nc.sync.dma_start(w2_s[:], w2_d[:])
            wmain_s = constp.tile([67, IPC * slot], DT)
            nc.sync.dma_start(wmain_s[:], wmain_d[:])
            bias_h_s = constp.tile([CH, 1], F32)
            nc.sync.dma_start(bias_h_s[:], bias_h_d[:])
            bias_mf_s = constp.tile([CH, 1], F32)
            nc.sync.dma_start(bias_mf_s[:], bias_mf_d[:])

            for il in range(IPC):
                # ---- load padded inputs for the image ----
                fhm_t = finp.tile([128, 2 * PHW16], DT, tag="fhm")
                for j in range(2):
                    nc.sync.dma_start(
                        fhm_t[:, j * PHW16:(j + 1) * PHW16], fhm_d[il, j]
                    )
                fm_t = finp.tile([128, PHW8], DT, tag="fm")
                nc.sync.dma_start(fm_t[:], fm_d[il])

                fhm_v = fhm_t[:].rearrange("q (b r c) -> q b r c", b=2, r=PH16)
                fm_v = fm_t[:].rearrange("q (r c) -> q r c", r=PH8)

                # ---- heatmap branch: 3x3 conv + ReLU ----
                h_t = hp.tile([CH, HW16], DT)
                for c in range(NC16):
                    ps = cpsp.tile([128, PSB], F32, tag="cps")
                    for j in range(2):  # cin chunks
                        for t, (ky, kx) in enumerate(TAPS):
                            rhs = fhm_v[
                                :, j,
                                R16 * c + ky: R16 * c + ky + R16,
                                kx: kx + W16,
                            ]
                            nc.tensor.matmul(
                                ps[:CH, :CHK],
                                w1t_s[:, (j * 9 + t) * CH:(j * 9 + t + 1) * CH],
                                rhs,
                                start=(j == 0 and t == 0),
                                stop=(j == 1 and t == 8),
                            )
                    nc.scalar.activation(
                        h_t[:, c * CHK:(c + 1) * CHK],
                        ps[:CH, :CHK],
                        RELU,
                        bias=bias_h_s[:],
                    )

                # ---- heatmap branch: 1x1 conv ----
                hm_sb = stagep.tile([1, HW16], F32, tag="hmsb")
                for c in range(NC16):
                    hps_t = hpsp.tile([128, PSB], F32, tag="hps")
                    nc.tensor.matmul(
                        hps_t[:1, :CHK],
                        w2_s[:],
                        h_t[:, c * CHK:(c + 1) * CHK],
                        start=True,
                        stop=True,
                    )
                    nc.vector.tensor_copy(
                        hm_sb[:, c * CHK:(c + 1) * CHK], hps_t[:1, :CHK]
                    )
                nc.sync.dma_start(hm_out_d[il:il + 1, :], hm_sb[:])

                # ---- mask branch: 3x3 conv + ReLU ----
                # rows 0-63: mf; rows 64-66: coord_x / coord_y / ones, so the
                # dynamic head is a single K=67 contraction.
                mf_t = mfp.tile([67, HW8], DT)
                nc.sync.dma_start(mf_t[64:67, :], coords_d[:])
                for c in range(NC8):
                    ps = cpsp.tile([128, PSB], F32, tag="cps")
                    for t, (ky, kx) in enumerate(TAPS):
                        rhs = fm_v[
                            :,
                            R8 * c + ky: R8 * c + ky + R8,
                            kx: kx + W8,
                        ]
                        nc.tensor.matmul(
                            ps[:CH, :CHK],
                            wmt_s[:, t * CH:(t + 1) * CH],
                            rhs,
                            start=(t == 0),
                            stop=(t == 8),
                        )
                    nc.scalar.activation(
                        mf_t[:CH, c * CHK:(c + 1) * CHK],
                        ps[:CH, :CHK],
                        RELU,
                        bias=bias_mf_s[:],
                    )

                # ---- dynamic heads (single K=67 matmul per chunk) ----
                dyn_sb = stagep.tile([slot, HW8], F32, tag="dynsb")
                for c in range(NC8):
                    dps_t = dpsp.tile([128, PSB], F32, tag="dps")
                    nc.tensor.matmul(
                        dps_t[:slot, :CHK],
                        wmain_s[:, il * slot:(il + 1) * slot],
                        mf_t[:, c * CHK:(c + 1) * CHK],
                        start=True,
                        stop=True,
                    )
                    nc.vector.tensor_copy(
                        dyn_sb[:, c * CHK:(c + 1) * CHK], dps_t[:slot, :CHK]
                    )
                nc.sync.dma_start(dyn_out_d[il], dyn_sb[:, :])

    nc.finalize()
    return nc


def _build_bf16(slot: int) -> bass.Bass:
    """bf16 variant: image pairs packed into PE column halves (col tiling,
    concurrent matmuls) for convs and dynamic heads. All matmul data bf16
    (error ~4e-3 rel, dominated by input rounding on the heatmap branch);
    PSUM accumulation stays fp32.

    Schedule: both pairs' heatmap convs first (small input, lands first),
    then per pair: 1x1 conv, mask conv with lag-1 pipelined dynamic heads.
    Heatmap inputs ride the scalar HWDGE ring, mask inputs the sync ring.
    """
    BF = mybir.dt.bfloat16
    nc = bacc.Bacc("TRN2", target_bir_lowering=False, debug=False)

    fhm_d = nc.declare_dram_parameter("fhm", [IPC, 2, 128, PHW16], BF, isOutput=False)
    fm_d = nc.declare_dram_parameter("fmask", [IPC, 128, PHW8], BF, isOutput=False)
    w1t_d = nc.declare_dram_parameter("w1t", [128, 18 * CH], BF, isOutput=False)
    wmt_d = nc.declare_dram_parameter("wmt", [128, 9 * CH], BF, isOutput=False)
    w2_d = nc.declare_dram_parameter("w2", [128, 1], BF, isOutput=False)
    coords_d = nc.declare_dram_parameter("coords", [3, HW8], BF, isOutput=False)
    wmain_d = nc.declare_dram_parameter("wmain", [67, IPC * slot], BF, isOutput=False)
    bias_h_d = nc.declare_dram_parameter("bias_h", [CH, 1], F32, isOutput=False)
    bias_mf_d = nc.declare_dram_parameter("bias_mf", [CH, 1], F32, isOutput=False)

    hm_out_d = nc.declare_dram_parameter("hm_out", [IPC, HW16], F32, isOutput=True)
    dyn_out_d = nc.declare_dram_parameter("dyn_out", [IPC, slot, HW8], F32, isOutput=True)

    RELU = mybir.ActivationFunctionType.Relu

    with tile.TileContext(nc) as tc:
        with (
            tc.tile_pool(name="const", bufs=1) as constp,
            tc.tile_pool(name="fin", bufs=2) as finp,
            tc.tile_pool(name="mf", bufs=2) as mfp,
            tc.tile_pool(name="h", bufs=2) as hp,
            tc.tile_pool(name="stage", bufs=2) as stagep,
            tc.tile_pool(name="cps", bufs=4, space="PSUM") as cpsp,
            tc.tile_pool(name="dps", bufs=2, space="PSUM") as dpsp,
            tc.tile_pool(name="hps", bufs=2, space="PSUM") as hpsp,
        ):
            fhm_tiles, fm_tiles, h_tiles = [], [], []
            for p in range(2):
                fhm_t = finp.tile([128, 4 * PHW16], BF, name=f"fhm{p}", tag="fhm")
                fm_t = finp.tile([128, 2 * PHW8], BF, name=f"fm{p}", tag="fm")
                fhm_tiles.append(fhm_t)
                fm_tiles.append(fm_t)
            w1t_s = constp.tile([128, 18 * CH], BF)
            nc.sync.dma_start(w1t_s[:], w1t_d[:])
            nc.scalar.dma_start(
                fhm_tiles[0][:].rearrange("q (b f) -> q b f", b=4),
                fhm_d[0:2].rearrange("a b q f -> q (a b) f"),
            )
            nc.scalar.dma_start(
                fhm_tiles[1][:].rearrange("q (b f) -> q b f", b=4),
                fhm_d[2:4].rearrange("a b q f -> q (a b) f"),
            )
            nc.sync.dma_start(
                fm_tiles[0][:].rearrange("q (i f) -> q i f", i=2),
                fm_d[0:2].rearrange("a q f -> q a f"),
            )
            nc.sync.dma_start(
                fm_tiles[1][:].rearrange("q (i f) -> q i f", i=2),
                fm_d[2:4].rearrange("a q f -> q a f"),
            )
            wmt_s = constp.tile([128, 9 * CH], BF)
            nc.sync.dma_start(wmt_s[:], wmt_d[:])
            w2_s = constp.tile([128, 1], BF)
            nc.scalar.dma_start(w2_s[:], w2_d[:])
            wmain_s = constp.tile([67, IPC * slot], BF)
            nc.scalar.dma_start(wmain_s[:], wmain_d[:])
            bias_h_s = constp.tile([CH, 1], F32)
            nc.scalar.dma_start(bias_h_s[:], bias_h_d[:])
            bias_mf_s = constp.tile([CH, 1], F32)
            nc.scalar.dma_start(bias_mf_s[:], bias_mf_d[:])

            # ================== heatmap 3x3 convs, both pairs ==============
            for p in range(2):
                fhm_v = fhm_tiles[p][:].rearrange("q (b r c) -> q b r c", b=4, r=PH16)
                h_t = hp.tile([128, HW16], BF)
                for c in range(NC16):
                    ps2 = [cpsp.tile([128, PSB], F32, name=f"cps{i_}", tag="cps")
                           for i_ in range(2)]
                    for j in range(2):
                        for t, (ky, kx) in enumerate(TAPS):
                            for i in range(2):
                                rhs = fhm_v[
                                    :, i * 2 + j,
                                    R16 * c + ky: R16 * c + ky + R16,
                                    kx: kx + W16,
                                ]
                                nc.tensor.matmul(
                                    ps2[i][i * CH:(i + 1) * CH, :CHK],
                                    w1t_s[:, (j * 9 + t) * CH:(j * 9 + t + 1) * CH],
                                    rhs,
                                    start=(j == 0 and t == 0),
                                    stop=(j == 1 and t == 8),
                                )
                    for i in range(2):
                        nc.scalar.activation(
                            h_t[i * CH:(i + 1) * CH, c * CHK:(c + 1) * CHK],
                            ps2[i][i * CH:(i + 1) * CH, :CHK],
                            RELU,
                            bias=bias_h_s[:],
                        )
                h_tiles.append(h_t)

            # ============= per pair: 1x1, mask conv, dynamic heads =========
            for p in range(2):
                # heatmap 1x1 (h ReLUs finished during the hm phase)
                h_t = h_tiles[p]
                hm_sb = stagep.tile([64, HW16], F32, tag="hmsb")
                for c in range(NC16):
                    hps_t = hpsp.tile([128, PSB], F32, tag="hps")
                    for i in range(2):
                        nc.tensor.matmul(
                            hps_t[i * 32:i * 32 + 1, :CHK],
                            w2_s[i * CH:(i + 1) * CH, :],
                            h_t[i * CH:(i + 1) * CH, c * CHK:(c + 1) * CHK],
                            start=True,
                            stop=True,
                        )
                    for i in range(2):
                        nc.vector.tensor_copy(
                            hm_sb[i * 32:i * 32 + 1, c * CHK:(c + 1) * CHK],
                            hps_t[i * 32:i * 32 + 1, :CHK],
                        )
                for i in range(2):
                    nc.sync.dma_start(
                        hm_out_d[2 * p + i:2 * p + i + 1, :],
                        hm_sb[i * 32:i * 32 + 1, :],
                    )

                # mask conv: per-image mf tiles (rows 0-63 mf, 64-66 coords)
                # so the dynamic head is one K=67 matmul per chunk. Image B's
                # ReLU shifts partitions 64-127 -> 0-63.
                fm_v = fm_tiles[p][:].rearrange("q (b r c) -> q b r c", b=2, r=PH8)
                mf2 = []
                for i in range(2):
                    mf_t = mfp.tile([67, HW8], BF, name=f"mf{i}", tag=f"mf{i}")
                    nc.sync.dma_start(mf_t[64:67, :], coords_d[:])
                    mf2.append(mf_t)
                dyn_sb = stagep.tile([2 * slot, HW8], F32, tag="dynsb")

                def emit_dyn(c):
                    # both images' heads in one bank: A at cols 0-31,
                    # B at cols 32-63 (sequential single-matmul groups)
                    dps_t = dpsp.tile([128, PSB], F32, tag="dps")
                    for i in range(2):
                        nc.tensor.matmul(
                            dps_t[i * slot:(i + 1) * slot, :CHK],
                            wmain_s[:, (2 * p + i) * slot:(2 * p + i + 1) * slot],
                            mf2[i][:, c * CHK:(c + 1) * CHK],
                            start=True,
                            stop=True,
                        )
                    nc.vector.tensor_copy(
                        dyn_sb[:, c * CHK:(c + 1) * CHK],
                        dps_t[:2 * slot, :CHK],
                    )

                for c in range(NC8):
                    ps2 = [cpsp.tile([128, PSB], F32, name=f"cpsm{i_}", tag="cps")
                           for i_ in range(2)]
                    for t, (ky, kx) in enumerate(TAPS):
                        for i in range(2):
                            rhs = fm_v[
                                :, i,
                                R8 * c + ky: R8 * c + ky + R8,
                                kx: kx + W8,
                            ]
                            nc.tensor.matmul(
                                ps2[i][i * CH:(i + 1) * CH, :CHK],
                                wmt_s[:, t * CH:(t + 1) * CH],
                                rhs,
                                start=(t == 0),
                                stop=(t == 8),
                            )
                    for i in range(2):
                        nc.scalar.activation(
                            mf2[i][:CH, c * CHK:(c + 1) * CHK],
                            ps2[i][i * CH:(i + 1) * CH, :CHK],
                            RELU,
                            bias=bias_mf_s[:],
                        )
                    if c >= 1:
                        emit_dyn(c - 1)
                    if c == NC8 - 1:
                        # first half of the outputs can ship early
                        for i in range(2):
                            nc.sync.dma_start(
                                dyn_out_d[2 * p + i, :, :3 * CHK],
                                dyn_sb[i * slot:(i + 1) * slot, :3 * CHK],
                            )
                emit_dyn(NC8 - 1)
                for i in range(2):
                    nc.sync.dma_start(
                        dyn_out_d[2 * p + i, :, 3 * CHK:],
                        dyn_sb[i * slot:(i + 1) * slot, 3 * CHK:],
                    )

    nc.finalize()
    return nc


def _prepare(f_hm, f_mask, dyn_params, inst_idx, hm_w1, hm_b1, hm_w2, hm_b2,
             mask_w, mask_b, variant="f32r"):
    """Host-side sharding + packing. Returns (in_maps, slot, inst_map)."""
    f_hm = np.asarray(f_hm, np.float32)
    f_mask = np.asarray(f_mask, np.float32)
    dyn_params = np.asarray(dyn_params, np.float32)
    inst_idx = np.asarray(inst_idx).astype(np.int64)
    hm_w1 = np.asarray(hm_w1, np.float32)
    hm_b1 = np.asarray(hm_b1, np.float32)
    mask_w = np.asarray(mask_w, np.float32)
    mask_b = np.asarray(mask_b, np.float32)

    n_inst = inst_idx.shape[0]
    counts = np.bincount(inst_idx, minlength=B)
    kmax = int(counts.max())
    half = max(16, -(-kmax // 16) * 16)  # per-head slot count, multiple of 16
    slot = 2 * half
    if slot > 128:
        raise ValueError(f"too many instances on one image: {kmax}")

    # padded conv inputs (zero borders baked in -> contiguous DMAs)
    fhm_pad = np.zeros((B, 2, 128, PH16, PW16), np.float32)
    fhm_pad[:, :, :, 1:1 + H16, 1:1 + W16] = f_hm.reshape(B, 2, 128, H16, W16)
    fhm_pad = fhm_pad.reshape(NCORES, IPC, 2, 128, PHW16)
    fm_pad = np.zeros((B, 128, PH8, PW8), np.float32)
    fm_pad[:, :, 1:1 + H8, 1:1 + W8] = f_mask
    fm_pad = fm_pad.reshape(NCORES, IPC, 128, PHW8)

    # conv weights as lhsT tap matrices, packed in SBUF layout [K=128, taps*M]
    w1t = np.ascontiguousarray(
        hm_w1.reshape(CH, 2, 128, 3, 3).transpose(2, 1, 3, 4, 0).reshape(128, 18 * CH)
    )
    wmt = np.ascontiguousarray(
        mask_w.transpose(1, 2, 3, 0).reshape(128, 9 * CH)
    )
    w2 = np.ascontiguousarray(np.asarray(hm_w2, np.float32).reshape(1, CH).T)
    if variant == "bf16":
        import ml_dtypes
        bf = ml_dtypes.bfloat16
        fhm_pad = fhm_pad.astype(bf)
        fm_pad = fm_pad.astype(bf)
        w1t = w1t.astype(bf)
        wmt = wmt.astype(bf)
        w2 = np.concatenate([w2, w2], axis=0).astype(bf)

    coords = np.stack([
        np.tile(np.arange(W8, dtype=np.float32), H8),
        np.repeat(np.arange(H8, dtype=np.float32), W8),
        np.ones(HW8, np.float32),
    ])

    bias_h = hm_b1[:, None].astype(np.float32)
    bias_mf = mask_b[:, None].astype(np.float32)

    # dynamic params routed to owning (core, image) and packed into slots.
    # rows 0-63: mf-channel weights; rows 64-66: coord_x / coord_y / bias.
    wmain = np.zeros((NCORES, 67, IPC * slot), np.float32)
    inst_map = np.zeros((n_inst, 3), np.int64)  # (core, img_local, slot_i)
    for bimg in range(B):
        k, il = divmod(bimg, IPC)
        ns = np.nonzero(inst_idx == bimg)[0]
        for si, n in enumerate(ns):
            col = il * slot + si
            wmain[k, :CH, col] = dyn_params[n, 2:66]
            wmain[k, 64:67, col] = (
                dyn_params[n, 0], dyn_params[n, 1], dyn_params[n, 66] - 2.19
            )
            col = il * slot + half + si
            wmain[k, :CH, col] = dyn_params[n, 69:133]
            wmain[k, 64:67, col] = (
                dyn_params[n, 67], dyn_params[n, 68], dyn_params[n, 133]
            )
            inst_map[n] = (k, il, si)

    if variant == "bf16":
        import ml_dtypes
        coords = coords.astype(ml_dtypes.bfloat16)
        wmain = wmain.astype(ml_dtypes.bfloat16)
    in_maps = []
    for k in range(NCORES):
        in_maps.append({
            "fhm": np.ascontiguousarray(fhm_pad[k]),
            "fmask": np.ascontiguousarray(fm_pad[k]),
            "w1t": w1t,
            "wmt": wmt,
            "w2": w2,
            "coords": np.ascontiguousarray(coords),
            "wmain": np.ascontiguousarray(wmain[k]),
            "bias_h": bias_h,
            "bias_mf": bias_mf,
        })
    return in_maps, slot, inst_map


def _postprocess(results, slot, inst_map, hm_b2):
    half = slot // 2
    hm = np.stack([r["hm_out"] for r in results])          # [8, IPC, 1000]
    hm = hm.reshape(B, 1, H16, W16) + np.float32(np.asarray(hm_b2).reshape(()))
    n_inst = inst_map.shape[0]
    masks = np.empty((n_inst, 1, H8, W8), np.float32)
    regs = np.empty((n_inst, 1, H8, W8), np.float32)
    for n in range(n_inst):
        k, il, si = inst_map[n]
        dyn = results[k]["dyn_out"]                        # [IPC, slot, 4000]
        masks[n, 0] = dyn[il, si].reshape(H8, W8)
        regs[n, 0] = dyn[il, half + si].reshape(H8, W8)
    return hm.astype(np.float32), masks, regs


LAST_RESULT = None


def kernel(f_hm, f_mask, dyn_params, inst_idx, hm_w1, hm_b1, hm_w2, hm_b2,
           mask_w, mask_b):
    global LAST_RESULT
    in_maps, slot, inst_map = _prepare(
        f_hm, f_mask, dyn_params, inst_idx, hm_w1, hm_b1, hm_w2, hm_b2,
        mask_w, mask_b, variant=VARIANT,
    )
    key = (slot, VARIANT, MM_DTYPE)
    if key not in _PROGRAM_CACHE:
        if VARIANT == "bf16":
            _PROGRAM_CACHE[key] = _build_bf16(slot)
        else:
            _PROGRAM_CACHE[key] = _build(slot, MM_DTYPE)
    nc = _PROGRAM_CACHE[key]
    trace = bool(int(os.environ.get("KERNEL_TRACE", "0")))
    res = run_bass_kernel_spmd(nc, in_maps, list(range(NCORES)), trace=trace)
    LAST_RESULT = res
    return _postprocess(res.results, slot, inst_map, hm_b2)
